# revision 1
# baseline (speedup 1.0000x reference)
"""Trainium2 Bass kernel for per-sample multi-head Linear (MoE-style routing).

Computes logits[i] = x[i] @ W[system_id[i]].T + b[system_id[i]] for
x:[B,D]=[262144,256], W:[S,C,D]=[16,10,256], b:[S,C], int system ids.

Strategy: data-parallel over 8 NeuronCores (32768 rows each). Each core:
  - streams x^T tiles (host pre-transposed so D sits on SBUF partitions),
  - computes the dense all-system logits [128b, 160sc] with two fp32r
    matmuls (k=128 each) + a rank-1 bias matmul into PSUM,
  - selects each row's own head with a fused (iota==sid)*logits multiply
    on DVE followed by a segmented reduce over the 16 systems,
  - writes [128,10] results back, batched per 1024-row chunk.
"""

import sys
import numpy as np

if "/opt/trn_rl_repo" not in sys.path:
    sys.path.insert(0, "/opt/trn_rl_repo")

import concourse.bacc as bacc
import concourse.bass as bass
import concourse.mybir as mybir
import concourse.tile as tile
from concourse.bass_utils import run_bass_kernel_spmd

B = 262144
D = 256
S = 16
C = 10
N_CORES = 8
B_CORE = B // N_CORES  # 32768

SC = S * C           # 160
SC_PAD = 256         # pad matmul free dim to 256 -> fp32r runs 1 cyc/row
CHUNK_B = 1024       # rows per DMA chunk
SUB_B = 128          # rows per matmul subtile
SUBS = CHUNK_B // SUB_B

F32 = mybir.dt.float32
F32R = mybir.dt.float32r


def build_nc(
    n_rows: int = B_CORE,
    chunk_b: int = CHUNK_B,
    xt_bufs: int = 3,
    psum_bufs: int = 4,
    spread_dma: bool = True,
    use_bias_mm: bool = True,
    act_copy: bool = True,
    repeat: int = 1,
    batch_mm: bool = False,
):
    """Build the per-core Bass program. Same program runs SPMD on all cores."""
    assert n_rows % chunk_b == 0 and chunk_b % SUB_B == 0
    n_chunks = n_rows // chunk_b
    subs = chunk_b // SUB_B
    n_tiles = n_rows // SUB_B

    nc = bacc.Bacc(
        "TRN2",
        target_bir_lowering=False,
        debug=False,
        num_devices=N_CORES,
    )

    xT = nc.dram_tensor("xT", [D, n_rows], F32R, kind="ExternalInput")
    sid = nc.dram_tensor("sid", [SUB_B, n_tiles], F32, kind="ExternalInput")
    wt = nc.dram_tensor("wt", [D, SC_PAD], F32R, kind="ExternalInput")
    biasrow = nc.dram_tensor("biasrow", [1, SC_PAD], F32R, kind="ExternalInput")
    ones = nc.dram_tensor("ones", [1, SUB_B], F32R, kind="ExternalInput")
    # sfull[p, c*S + s] = s  (iota over systems, repeated per class)
    sfull = nc.dram_tensor("sfull", [SUB_B, C * S], F32, kind="ExternalInput")
    out = nc.dram_tensor("out", [n_rows, C], F32, kind="ExternalOutput")

    with tile.TileContext(nc) as tc:
        with (
            tc.tile_pool(name="consts", bufs=1) as consts,
            tc.tile_pool(name="xtp0", bufs=xt_bufs) as xtp0,
            tc.tile_pool(name="xtp1", bufs=xt_bufs) as xtp1,
            tc.tile_pool(name="alp", bufs=4) as alp,
            tc.tile_pool(name="prodp", bufs=4) as prodp,
            tc.tile_pool(name="outp", bufs=3) as outp,
            tc.tile_pool(name="psum", bufs=psum_bufs, space=bass.MemorySpace.PSUM) as psump,
        ):
            wt0 = consts.tile([SUB_B, SC_PAD], F32R, tag="wt0")
            wt1 = consts.tile([SUB_B, SC_PAD], F32R, tag="wt1")
            bias_t = consts.tile([1, SC_PAD], F32R, tag="bias")
            ones_t = consts.tile([1, SUB_B], F32R, tag="ones")
            sfull_t = consts.tile([SUB_B, C * S], F32, tag="sfull")
            sid_t = consts.tile([SUB_B, n_tiles], F32, tag="sid")

            nc.sync.dma_start(wt0[:], wt[0:SUB_B, :])
            nc.sync.dma_start(wt1[:], wt[SUB_B : 2 * SUB_B, :])
            nc.sync.dma_start(bias_t[:], biasrow[:])
            nc.sync.dma_start(ones_t[:], ones[:])
            nc.sync.dma_start(sfull_t[:], sfull[:])
            nc.sync.dma_start(sid_t[:], sid[:])

            out_r = out.rearrange("(n j p) c -> n p j c", p=SUB_B, j=subs)

            for ci_rep in range(n_chunks * repeat):
                ci = ci_rep % n_chunks
                xt0 = xtp0.tile([SUB_B, chunk_b], F32R, tag="xt0")
                xt1 = xtp1.tile([SUB_B, chunk_b], F32R, tag="xt1")
                c0 = ci * chunk_b
                eng1 = nc.gpsimd if spread_dma else nc.sync
                nc.sync.dma_start(xt0[:], xT[0:SUB_B, c0 : c0 + chunk_b])
                eng1.dma_start(xt1[:], xT[SUB_B : 2 * SUB_B, c0 : c0 + chunk_b])

                outb = outp.tile([SUB_B, subs, C], F32, tag="outb")

                def emit_mm(j, ps):
                    js = j * SUB_B
                    nc.tensor.matmul(
                        ps[:], xt0[:, js : js + SUB_B], wt0[:], start=True, stop=False
                    )
                    nc.tensor.matmul(
                        ps[:],
                        xt1[:, js : js + SUB_B],
                        wt1[:],
                        start=False,
                        stop=not use_bias_mm,
                    )
                    if use_bias_mm:
                        nc.tensor.matmul(
                            ps[:], ones_t[:], bias_t[:], start=False, stop=True
                        )

                def emit_select(j, ps):
                    t = ci * subs + j
                    # prod[p, c, s] = (sfull[p,c,s] == sid[p,t]) * al[p, s*C + c]
                    prod = prodp.tile([SUB_B, C, S], F32, tag="prod")
                    if act_copy:
                        # ACT copies the 160 real logits out of PSUM.
                        al = alp.tile([SUB_B, SC], F32, tag="al")
                        nc.scalar.copy(al[:], ps[:, 0:SC])
                        al_cs = al[:].rearrange("p (s c) -> p c s", s=S, c=C)
                    else:
                        al_cs = ps[:, 0:SC].rearrange("p (s c) -> p c s", s=S, c=C)
                    nc.vector.scalar_tensor_tensor(
                        out=prod[:],
                        in0=sfull_t[:].rearrange("p (c s) -> p c s", c=C, s=S),
                        scalar=sid_t[:, t : t + 1],
                        in1=al_cs,
                        op0=mybir.AluOpType.is_equal,
                        op1=mybir.AluOpType.mult,
                    )
                    # sel[p, c] = sum_s prod[p, c, s]
                    nc.vector.tensor_reduce(
                        out=outb[:, j, :],
                        in_=prod[:],
                        axis=mybir.AxisListType.X,
                        op=mybir.AluOpType.add,
                    )

                if batch_mm:
                    pss = []
                    for j in range(subs):
                        ps = psump.tile([SUB_B, SC_PAD], F32, tag="ps")
                        emit_mm(j, ps)
                        pss.append(ps)
                    for j in range(subs):
                        emit_select(j, pss[j])
                else:
                    for j in range(subs):
                        ps = psump.tile([SUB_B, SC_PAD], F32, tag="ps")
                        emit_mm(j, ps)
                        emit_select(j, ps)

                nc.sync.dma_start(out_r[ci], outb[:])

    nc.compile()
    return nc


def _round_fp32r(a):
    """Round fp32 -> fp32r (round-to-nearest-even at 13 dropped mantissa bits),
    matching walrus's fp32_to_fp32r. Matmul operands are consumed at this
    precision by the PE, so pre-rounding keeps host/sim/HW consistent."""
    bits = a.astype(np.float32).view(np.uint32)
    lsb = (bits >> np.uint32(13)) & np.uint32(1)
    rounded = (bits + np.uint32(0x0FFF) + lsb) & np.uint32(0xFFFFE000)
    return rounded.view(np.float32)


def _host_prep(x, system_id, W, b):
    """Host-side layout prep shared by all cores (weights) and per-core (x/sid)."""
    wt = np.zeros((D, SC_PAD), dtype=np.float32)
    wt[:, :SC] = _round_fp32r(W.reshape(SC, D).T)
    biasrow = np.zeros((1, SC_PAD), dtype=np.float32)
    biasrow[0, :SC] = _round_fp32r(b.reshape(SC))
    ones = np.ones((1, SUB_B), dtype=np.float32)
    sfull = np.tile(
        np.repeat(np.arange(S, dtype=np.float32)[None, :], C, axis=0).reshape(1, C * S),
        (SUB_B, 1),
    )
    return wt, biasrow, ones, sfull


_NC_CACHE = {}


def kernel(x, system_id, W, b):
    x = np.asarray(x, dtype=np.float32)
    system_id = np.asarray(system_id)
    W = np.asarray(W, dtype=np.float32)
    b = np.asarray(b, dtype=np.float32)

    key = (x.shape[0],)
    if key not in _NC_CACHE:
        _NC_CACHE[key] = build_nc(x.shape[0] // N_CORES)
    nc = _NC_CACHE[key]

    wt, biasrow, ones, sfull = _host_prep(x, system_id, W, b)

    n_rows = x.shape[0] // N_CORES
    n_tiles = n_rows // SUB_B
    in_maps = []
    for core in range(N_CORES):
        lo, hi = core * n_rows, (core + 1) * n_rows
        xT_shard = np.ascontiguousarray(x[lo:hi].T)  # [D, n_rows]
        sid_shard = np.ascontiguousarray(
            system_id[lo:hi].astype(np.float32).reshape(n_tiles, SUB_B).T
        )  # [128, n_tiles]
        in_maps.append(
            {
                "xT": xT_shard,
                "sid": sid_shard,
                "wt": wt,
                "biasrow": biasrow,
                "ones": ones,
                "sfull": sfull,
            }
        )

    res = run_bass_kernel_spmd(nc, in_maps, core_ids=list(range(N_CORES)))
    out = np.concatenate([res.results[i]["out"] for i in range(N_CORES)], axis=0)
    return out.astype(np.float32)



# revision 4
# speedup vs baseline: 25.3704x; 25.3704x over previous
"""Trainium2 Bass kernel for per-sample multi-head Linear (MoE-style routing).

Computes logits[i] = x[i] @ W[system_id[i]].T + b[system_id[i]] for
x:[B,D]=[262144,256], W:[S,C,D]=[16,10,256], b:[S,C], int system ids.

Strategy: data-parallel over 8 NeuronCores (32768 rows each). Each core:
  - streams x^T tiles in bf16 (host pre-transposes and rounds; D sits on
    SBUF partitions) -- halves HBM traffic vs fp32,
  - computes the dense all-system logits [128b, 160sc] per 128-row
    subtile with two bf16 matmuls (k=128 each, x-tile stationary with
    FWL) accumulated on top of a rank-1 bias matmul whose `ones`
    stationary is hoisted across each group of subtiles,
  - selects each row's own head with a fused (iota==sid)*logits multiply
    on DVE (all-contiguous bf16, class-major layout) followed by a
    segmented reduce over the 16 systems (alternating DVE / GpSimd),
  - writes [2048,10] fp32 results back per x-tile.
"""

import sys
import numpy as np

if "/opt/trn_rl_repo" not in sys.path:
    sys.path.insert(0, "/opt/trn_rl_repo")

import concourse.bacc as bacc
import concourse.bass as bass
import concourse.mybir as mybir
import concourse.tile as tile
from concourse.bass_utils import run_bass_kernel_spmd

B = 262144
D = 256
S = 16
C = 10
N_CORES = 8
B_CORE = B // N_CORES  # 32768

SC = S * C   # 160
SUB_B = 128  # rows per matmul subtile

F32 = mybir.dt.float32
BF16 = mybir.dt.bfloat16


def build_nc(
    n_rows: int = B_CORE,
    dma_b: int = 2048,    # rows per x DMA tile
    group: int = 4,       # subtiles per PSUM group (bias-stationary hoist)
    psum_bufs: int = 8,
    xt_bufs: int = 3,
    gp_reduce: bool = True,
):
    """Build the per-core Bass program. Same program runs SPMD on all cores."""
    assert n_rows % dma_b == 0 and dma_b % (group * SUB_B) == 0
    n_dma = n_rows // dma_b
    subs_per_dma = dma_b // SUB_B
    groups_per_dma = subs_per_dma // group
    n_tiles = n_rows // SUB_B

    nc = bacc.Bacc(
        "TRN2",
        target_bir_lowering=False,
        debug=False,
        num_devices=N_CORES,
    )

    xT = nc.dram_tensor("xT", [D, n_rows], BF16, kind="ExternalInput")
    sid = nc.dram_tensor("sid", [SUB_B, n_tiles], BF16, kind="ExternalInput")
    # wt[d, c*S + s] = W[s, c, d]  (class-major so the select is contiguous)
    wt = nc.dram_tensor("wt", [D, SC], BF16, kind="ExternalInput")
    biasrow = nc.dram_tensor("biasrow", [1, SC], BF16, kind="ExternalInput")
    ones = nc.dram_tensor("ones", [1, SUB_B], BF16, kind="ExternalInput")
    # sfull[p, c*S + s] = s
    sfull = nc.dram_tensor("sfull", [SUB_B, SC], BF16, kind="ExternalInput")
    out = nc.dram_tensor("out", [n_rows, C], F32, kind="ExternalOutput")

    with tile.TileContext(nc) as tc:
        with (
            tc.tile_pool(name="consts", bufs=1) as consts,
            tc.tile_pool(name="xtp0", bufs=xt_bufs) as xtp0,
            tc.tile_pool(name="xtp1", bufs=xt_bufs) as xtp1,
            tc.tile_pool(name="alp", bufs=2 * group) as alp,
            tc.tile_pool(name="prodp", bufs=2 * group) as prodp,
            tc.tile_pool(name="outp", bufs=3) as outp,
            tc.tile_pool(name="psum", bufs=psum_bufs, space=bass.MemorySpace.PSUM) as psump,
        ):
            wt0 = consts.tile([SUB_B, SC], BF16, tag="wt0")
            wt1 = consts.tile([SUB_B, SC], BF16, tag="wt1")
            bias_t = consts.tile([1, SC], BF16, tag="bias")
            ones_t = consts.tile([1, SUB_B], BF16, tag="ones")
            sfull_t = consts.tile([SUB_B, SC], BF16, tag="sfull")
            sid_t = consts.tile([SUB_B, n_tiles], BF16, tag="sid")

            nc.sync.dma_start(wt0[:], wt[0:SUB_B, :])
            nc.sync.dma_start(wt1[:], wt[SUB_B : 2 * SUB_B, :])
            nc.sync.dma_start(bias_t[:], biasrow[:])
            nc.sync.dma_start(ones_t[:], ones[:])
            nc.sync.dma_start(sfull_t[:], sfull[:])
            nc.sync.dma_start(sid_t[:], sid[:])

            out_r = out.rearrange("(n j p) c -> n p j c", p=SUB_B, j=subs_per_dma)

            for di in range(n_dma):
                xt0 = xtp0.tile([SUB_B, dma_b], BF16, tag="xt0")
                xt1 = xtp1.tile([SUB_B, dma_b], BF16, tag="xt1")
                c0 = di * dma_b
                nc.sync.dma_start(xt0[:], xT[0:SUB_B, c0 : c0 + dma_b])
                nc.gpsimd.dma_start(xt1[:], xT[SUB_B : 2 * SUB_B, c0 : c0 + dma_b])

                outb = outp.tile([SUB_B, subs_per_dma, C], F32, tag="outb")

                for g in range(groups_per_dma):
                    pss = [
                        psump.tile([SUB_B, SC], F32, tag="ps", name="ps")
                        for _ in range(group)
                    ]
                    # Rank-1 bias matmuls share one `ones` stationary load.
                    for ps in pss:
                        nc.tensor.matmul(
                            ps[:], ones_t[:], bias_t[:], start=True, stop=False
                        )
                    for j, ps in enumerate(pss):
                        js = (g * group + j) * SUB_B
                        nc.tensor.matmul(
                            ps[:], xt0[:, js : js + SUB_B], wt0[:],
                            start=False, stop=False,
                        )
                        nc.tensor.matmul(
                            ps[:], xt1[:, js : js + SUB_B], wt1[:],
                            start=False, stop=True,
                        )
                    for j, ps in enumerate(pss):
                        jj = g * group + j
                        t = di * subs_per_dma + jj
                        al = alp.tile([SUB_B, SC], BF16, tag="al")
                        nc.scalar.copy(al[:], ps[:])
                        # prod[p, c, s] = (sfull[p,c,s] == sid[p,t]) * al[p, c*S+s]
                        prod = prodp.tile([SUB_B, SC], BF16, tag="prod")
                        nc.vector.scalar_tensor_tensor(
                            out=prod[:],
                            in0=sfull_t[:],
                            scalar=sid_t[:, t : t + 1],
                            in1=al[:],
                            op0=mybir.AluOpType.is_equal,
                            op1=mybir.AluOpType.mult,
                        )
                        # sel[p, c] = sum_s prod[p, c, s]
                        nc.vector.tensor_reduce(
                            out=outb[:, jj, :],
                            in_=prod[:].rearrange("p (c s) -> p c s", c=C, s=S),
                            axis=mybir.AxisListType.X,
                            op=mybir.AluOpType.add,
                        )

                nc.sync.dma_start(out_r[di], outb[:])

    nc.compile()
    return nc


def _round_bf16(a: np.ndarray) -> np.ndarray:
    """fp32 -> bf16 with round-to-nearest-even, returned as ml_dtypes.bfloat16."""
    import ml_dtypes

    bits = np.ascontiguousarray(a, dtype=np.float32).view(np.uint32)
    lsb = (bits >> np.uint32(16)) & np.uint32(1)
    rounded = ((bits + np.uint32(0x7FFF) + lsb) >> np.uint32(16)).astype(np.uint16)
    return rounded.view(ml_dtypes.bfloat16)


def _host_prep(x, system_id, W, b):
    """Host-side layout prep shared by all cores (weights) and per-core (x/sid)."""
    # wt[d, c*S + s] = W[s, c, d]
    wt = _round_bf16(np.transpose(W, (2, 1, 0)).reshape(D, SC))
    biasrow = _round_bf16(np.asarray(b, dtype=np.float32).T.reshape(1, SC))
    ones = _round_bf16(np.ones((1, SUB_B), dtype=np.float32))
    sfull = _round_bf16(
        np.tile(np.tile(np.arange(S, dtype=np.float32), C), (SUB_B, 1))
    )
    return wt, biasrow, ones, sfull


_NC_CACHE = {}


def kernel(x, system_id, W, b):
    x = np.asarray(x, dtype=np.float32)
    system_id = np.asarray(system_id)
    W = np.asarray(W, dtype=np.float32)
    b = np.asarray(b, dtype=np.float32)

    key = (x.shape[0],)
    if key not in _NC_CACHE:
        _NC_CACHE[key] = build_nc(x.shape[0] // N_CORES)
    nc = _NC_CACHE[key]

    wt, biasrow, ones, sfull = _host_prep(x, system_id, W, b)

    n_rows = x.shape[0] // N_CORES
    n_tiles = n_rows // SUB_B
    x_bf = _round_bf16(x)
    in_maps = []
    for core in range(N_CORES):
        lo, hi = core * n_rows, (core + 1) * n_rows
        xT_shard = np.ascontiguousarray(x_bf[lo:hi].T)  # [D, n_rows] bf16
        sid_shard = np.ascontiguousarray(
            _round_bf16(
                system_id[lo:hi].astype(np.float32).reshape(n_tiles, SUB_B).T
            )
        )  # [128, n_tiles] bf16
        in_maps.append(
            {
                "xT": xT_shard,
                "sid": sid_shard,
                "wt": wt,
                "biasrow": biasrow,
                "ones": ones,
                "sfull": sfull,
            }
        )

    res = run_bass_kernel_spmd(nc, in_maps, core_ids=list(range(N_CORES)))
    out = np.concatenate([res.results[i]["out"] for i in range(N_CORES)], axis=0)
    return out.astype(np.float32)


# revision 5
# speedup vs baseline: 40.2210x; 1.5854x over previous
"""Trainium2 Bass kernel for per-sample multi-head Linear (MoE-style routing).

Computes logits[i] = x[i] @ W[system_id[i]].T + b[system_id[i]] for
x:[B,D]=[262144,256], W:[S,C,D]=[16,10,256], b:[S,C], int system ids.

Strategy: data-parallel over 8 NeuronCores (32768 rows each), with the
per-row head selection folded into the matmul itself ("select-via-max"):

  ps[b, (c,s)] = x[b] @ Wt[:, (c,s)] + onehot[b] @ V[:, (c,s)]
  where V[k, (c,s)] = b[k,c] if s == k else -1e30

so every lane belonging to a head other than the row's own sits at ~-1e30
and the row's own lane holds the exact fp32 logit + bias. The selection is
then a single segmented reduce_max over the 16 systems -- no per-row mask
multiply, no separate bias add.

Per core, per 2048-row x-tile (bf16 throughout -> half the HBM traffic):
  - 3 matmuls per 128-row subtile (two k=128 halves of x, plus the onehot
    "penalty" matmul whose stationary is zero-padded to K=128 on device:
    mixing K=16 and K=128 stationaries stalls the PE pipeline ~3x),
  - PSUM packs 2 subtiles per bank [128, 320]; copies to SBUF alternate
    between the Scalar and Vector engines,
  - one reduce_max per 8 subtiles, output DMA issued from GpSimd.
"""

import sys
import numpy as np

if "/opt/trn_rl_repo" not in sys.path:
    sys.path.insert(0, "/opt/trn_rl_repo")

import concourse.bacc as bacc
import concourse.bass as bass
import concourse.mybir as mybir
import concourse.tile as tile
from concourse.bass_utils import run_bass_kernel_spmd

B = 262144
D = 256
S = 16
C = 10
N_CORES = 8
B_CORE = B // N_CORES  # 32768

SC = S * C   # 160
SUB_B = 128  # rows per matmul subtile

F32 = mybir.dt.float32
BF16 = mybir.dt.bfloat16


def build_nc(
    n_rows: int = B_CORE,
    dma_b: int = 2048,
    psum_bufs: int = 8,
    xt_bufs: int = 3,
    megap_bufs: int = 4,
    oh_bufs_n: int = 3,
    red_split: int = 2,
    alt_copy: bool = True,
):
    """Build the per-core Bass program. Same program runs SPMD on all cores."""
    assert n_rows % dma_b == 0
    n_dma = n_rows // dma_b
    subs_per_dma = dma_b // SUB_B      # 16
    packs_per_dma = subs_per_dma // 2  # 8
    assert packs_per_dma % red_split == 0

    nc = bacc.Bacc(
        "TRN2",
        target_bir_lowering=False,
        debug=False,
        num_devices=N_CORES,
    )

    xT = nc.dram_tensor("xT", [D, n_rows], BF16, kind="ExternalInput")
    oh = nc.dram_tensor("oh", [S, n_rows], BF16, kind="ExternalInput")
    # wt[d, c*S + s] = W[s, c, d]  (class-major, systems innermost)
    wt = nc.dram_tensor("wt", [D, SC], BF16, kind="ExternalInput")
    # vpen[k, c*S + s] = b[k, c] if s == k else -1e30
    vpen = nc.dram_tensor("vpen", [S, SC], BF16, kind="ExternalInput")
    out = nc.dram_tensor("out", [n_rows, C], F32, kind="ExternalOutput")

    with tile.TileContext(nc) as tc:
        with (
            tc.tile_pool(name="consts", bufs=1) as consts,
            tc.tile_pool(name="xtp0", bufs=xt_bufs) as xtp0,
            tc.tile_pool(name="xtp1", bufs=xt_bufs) as xtp1,
            tc.tile_pool(name="megap", bufs=megap_bufs) as megap,
            tc.tile_pool(name="outp", bufs=4) as outp,
            tc.tile_pool(name="psum", bufs=psum_bufs, space=bass.MemorySpace.PSUM) as psump,
        ):
            wt0 = consts.tile([SUB_B, SC], BF16, tag="wt0")
            wt1 = consts.tile([SUB_B, SC], BF16, tag="wt1")
            vpen_t = consts.tile([SUB_B, SC], BF16, tag="vpen")
            nc.sync.dma_start(wt0[:], wt[0:SUB_B, :])
            nc.sync.dma_start(wt1[:], wt[SUB_B : 2 * SUB_B, :])
            # vpen zero-padded to 128 partitions; rows 16..127 stay zero.
            nc.vector.memset(vpen_t[:], 0)
            nc.sync.dma_start(vpen_t[0:S, :], vpen[:])

            # Manually-cycled zero-padded onehot buffers (rows 16..127 stay 0)
            # so every matmul stationary is a uniform [128, 128] tile.
            oh_ts = []
            for i in range(oh_bufs_n):
                t = consts.tile(
                    [SUB_B, dma_b], BF16, tag=f"ohpad{i}", name=f"ohpad{i}"
                )
                nc.vector.memset(t[:], 0)
                oh_ts.append(t)

            out_r = out.rearrange("(n j p) c -> n p j c", p=SUB_B, j=subs_per_dma)

            for di in range(n_dma):
                xt0 = xtp0.tile([SUB_B, dma_b], BF16, tag="xt0")
                xt1 = xtp1.tile([SUB_B, dma_b], BF16, tag="xt1")
                oh_t = oh_ts[di % oh_bufs_n]
                c0 = di * dma_b
                nc.sync.dma_start(xt0[:], xT[0:SUB_B, c0 : c0 + dma_b])
                nc.sync.dma_start(xt1[:], xT[SUB_B : 2 * SUB_B, c0 : c0 + dma_b])
                nc.scalar.dma_start(oh_t[0:S, :], oh[:, c0 : c0 + dma_b])

                outb = outp.tile([SUB_B, subs_per_dma * C], F32, tag="outb")
                mega = megap.tile([SUB_B, subs_per_dma * SC], BF16, tag="mega")

                packs_per_red = packs_per_dma // red_split
                for pk in range(packs_per_dma):
                    ps = psump.tile([SUB_B, 2 * SC], F32, tag="ps", name="ps")
                    for h in range(2):
                        jj = pk * 2 + h
                        js = jj * SUB_B
                        lo, hi = h * SC, (h + 1) * SC
                        nc.tensor.matmul(
                            ps[:, lo:hi], xt0[:, js : js + SUB_B], wt0[:],
                            start=True, stop=False,
                        )
                        nc.tensor.matmul(
                            ps[:, lo:hi], xt1[:, js : js + SUB_B], wt1[:],
                            start=False, stop=False,
                        )
                        nc.tensor.matmul(
                            ps[:, lo:hi], oh_t[:, js : js + SUB_B], vpen_t[:],
                            start=False, stop=True,
                        )
                    if alt_copy and pk % 2 == 1:
                        nc.vector.tensor_copy(
                            mega[:, pk * 2 * SC : (pk + 1) * 2 * SC], ps[:]
                        )
                    else:
                        nc.scalar.copy(
                            mega[:, pk * 2 * SC : (pk + 1) * 2 * SC], ps[:]
                        )

                    if (pk + 1) % packs_per_red == 0:
                        r = pk // packs_per_red
                        m0 = r * packs_per_red * 2 * C
                        m1 = (r + 1) * packs_per_red * 2 * C
                        nc.vector.tensor_reduce(
                            out=outb[:, m0:m1],
                            in_=mega[:, m0 * S : m1 * S].rearrange(
                                "p (m s) -> p m s", m=m1 - m0, s=S
                            ),
                            axis=mybir.AxisListType.X,
                            op=mybir.AluOpType.max,
                        )
                nc.gpsimd.dma_start(
                    out_r[di],
                    outb[:].rearrange("p (j c) -> p j c", j=subs_per_dma, c=C),
                )

    nc.compile()
    return nc


def _round_bf16(a: np.ndarray) -> np.ndarray:
    """fp32 -> bf16 with round-to-nearest-even, returned as ml_dtypes.bfloat16."""
    import ml_dtypes

    bits = np.ascontiguousarray(a, dtype=np.float32).view(np.uint32)
    lsb = (bits >> np.uint32(16)) & np.uint32(1)
    rounded = ((bits + np.uint32(0x7FFF) + lsb) >> np.uint32(16)).astype(np.uint16)
    return rounded.view(ml_dtypes.bfloat16)


def _host_prep(W, b):
    """Weight-stack layout prep shared by all cores."""
    W = np.asarray(W, dtype=np.float32)
    b = np.asarray(b, dtype=np.float32)
    wt = _round_bf16(np.transpose(W, (2, 1, 0)).reshape(D, SC))
    vpen = np.full((S, SC), -1e30, dtype=np.float32)
    for k in range(S):
        vpen[k, np.arange(C) * S + k] = b[k]
    vpen = _round_bf16(vpen)
    return wt, vpen


_NC_CACHE = {}


def kernel(x, system_id, W, b):
    x = np.asarray(x, dtype=np.float32)
    system_id = np.asarray(system_id)

    key = (x.shape[0],)
    if key not in _NC_CACHE:
        _NC_CACHE[key] = build_nc(x.shape[0] // N_CORES)
    nc = _NC_CACHE[key]

    wt, vpen = _host_prep(W, b)

    n_rows = x.shape[0] // N_CORES
    x_bf = _round_bf16(x)
    eye = np.eye(S, dtype=np.float32)
    in_maps = []
    for core in range(N_CORES):
        lo, hi = core * n_rows, (core + 1) * n_rows
        in_maps.append(
            {
                "xT": np.ascontiguousarray(x_bf[lo:hi].T),           # [D, n_rows]
                "oh": np.ascontiguousarray(
                    _round_bf16(eye[:, system_id[lo:hi]])
                ),                                                    # [S, n_rows]
                "wt": wt,
                "vpen": vpen,
            }
        )

    res = run_bass_kernel_spmd(nc, in_maps, core_ids=list(range(N_CORES)))
    out = np.concatenate([res.results[i]["out"] for i in range(N_CORES)], axis=0)
    return out.astype(np.float32)
